# revision 4
# baseline (speedup 1.0000x reference)
"""CMFM loss kernel for Trainium2 (8 NeuronCores, Bass/Tile).

Math: for inputs f_v, f_a [B,T,D] with vn/an the D-normalized tensors,
  cos[b,t]    = s_va / (sqrt(s_vv)*sqrt(s_aa))          (per-timestep term)
  cross[i,j]  = (1/T) sum_t vn[i,t,:].an[j,t,:]
  sum_{i!=j} cross = (1/T)*(sum_t V_t.A_t  -  sum_{b,t} cos[b,t])
where V_t = sum_b vn[b,t,:], A_t = sum_b an[b,t,:].  So the BxB cross term
only needs the batch-summed normalized features -> data-parallel over B with
a tiny cross-core combine of the per-core partial V/A sums and cos stats.

v2 design (bf16 + TensorE offload):
  - Inputs cast to bf16 on host: halves DMA (8.4MB/core, ~27.5us roofline).
  - Per tile [128t, 256d]: ACT does Square+accum for s_vv (632ns); DVE does
    s_aa and s_va via scalar_tensor_tensor+accum (327ns each, STT has no
    fast perf modes so bf16 does not speed it, but it frees ACT).
  - The V/A scale-accumulate (was 2 STT/tile = 45us DVE) moves to the idle
    TensorEngine: V_acc[:,tc,:] += inv_v[t]*v[t,:] is a matmul with
    lhsT=diag(inv_v) [128,128] bf16, rhs=v chunk, accumulating over the 8
    b-rows in PSUM (bank tc holds V in [:,0,:] and A in [:,1,:]).
    has_written trick: V@b0 uses start=True (clears whole bank), A@b0 uses
    start=False and lands on cleared bits -> overwrite; all later matmuls
    accumulate; only A@b7 carries stop=True.  Diag tiles are built on DVE
    as tensor_scalar(eye_bf16, scalar=inv) (4x mode, ~94ns).
  - Eviction PSUM->SBUF on ACT (A-half then V-half per bank, single engine,
    so in-order execution plus the A@b7 dependency keeps PE writes and ACT
    reads off the same bank).
Outputs per core: cos stats [128,64], V_acc/A_acc [128,8*256] f32 partials.
Host: sums the 8 partial V/A tensors, dots them, applies label masks.

Runtime quirks discovered on this stack: InstTensorTensorReduce crashes
the NRT; Pool rejects TensorScalarPtr; ACT Rsqrt/Reciprocal banned in bass
(accuracy) -> Sqrt + DVE reciprocal.
"""

import numpy as np
import ml_dtypes

import concourse.bacc as bacc
import concourse.bass as bass
import concourse.tile as tile
from concourse import mybir
from concourse.bass_utils import run_bass_kernel_spmd

ALPHA, BETA, GAMMA = 2.0, 2.0, 1.0
B, T, D = 64, 1024, 256
N_CORES = 8
B_LOC = B // N_CORES          # 8 batch rows per core
P = 128                       # SBUF partitions
TCH = T // P                  # 8 t-chunks per batch row
NTILES = B_LOC * TCH          # 64 tiles per core

F32 = mybir.dt.float32
BF16 = mybir.dt.bfloat16
MULT = mybir.AluOpType.mult
ADD = mybir.AluOpType.add

import os as _os
# how many of the 8 sqa chunks per row run on ACT (rest: DVE STT). With the
# accumulates gone from DVE, DVE has slack -> default all sqa on DVE.
SQA_ACT_NUM, SQA_ACT_DEN = 0, 8
if _os.environ.get("K_SQA"):
    SQA_ACT_NUM, SQA_ACT_DEN = (int(x) for x in _os.environ["K_SQA"].split("/"))
K_EVICT = _os.environ.get("K_EVICT", "act")   # act | dve
K_DIAG_ACT = int(_os.environ.get("K_DIAG_ACT", "0"))  # 0..16 diags/row on ACT

_CACHE = {}
LAST_RESULTS = None


def _build_nc(repeat=1, loop_n=1):
    nc = bacc.Bacc("TRN2", debug=False)

    v = nc.dram_tensor("v", [B_LOC, T, D], BF16, kind="ExternalInput").ap()
    a = nc.dram_tensor("a", [B_LOC, T, D], BF16, kind="ExternalInput").ap()
    eye_in = nc.dram_tensor("eye", [P, P], BF16, kind="ExternalInput").ap()
    cos_out = nc.dram_tensor("cos_stat", [P, NTILES], F32, kind="ExternalOutput").ap()
    vacc_out = nc.dram_tensor("v_acc", [P, TCH * D], F32, kind="ExternalOutput").ap()
    aacc_out = nc.dram_tensor("a_acc", [P, TCH * D], F32, kind="ExternalOutput").ap()

    with tile.TileContext(nc) as tc:
        with (
            tc.tile_pool(name="io", bufs=int(_os.environ.get("K_IO_BUFS", "4"))) as io_pool,
            tc.tile_pool(name="scratch", bufs=6) as scratch,
            tc.tile_pool(name="small", bufs=12) as small,
            tc.tile_pool(name="diagp", bufs=6) as diagp,
            tc.tile_pool(name="acc", bufs=1) as accp,
            tc.psum_pool(name="ps", bufs=1) as psp,
        ):
            sva_stat = accp.tile([P, NTILES], F32)     # col = b*TCH+tc
            inv_stat = accp.tile([P, NTILES, 2], F32)  # (1/|v|, 1/|a|)
            cos_stat = accp.tile([P, NTILES], F32)
            vout = accp.tile([P, TCH, D], F32)
            aout = accp.tile([P, TCH, D], F32)
            eye_sb = accp.tile([P, P], BF16)
            nc.sync.dma_start(out=eye_sb[:], in_=eye_in)

            # one PSUM bank per t-chunk: [:,0,:]=V accum, [:,1,:]=A accum
            psum = [psp.tile([P, 2, D], F32, name=f"ps{i}") for i in range(TCH)]

            import contextlib
            loop_ctx = (
                tc.For_i(
                    0, loop_n, 1,
                    hint_engines=(
                        mybir.EngineType.DVE,
                        mybir.EngineType.Activation,
                        mybir.EngineType.SP,
                        mybir.EngineType.PE,
                    ),
                )
                if loop_n > 1
                else contextlib.nullcontext()
            )
            GRP = 4  # tiles per pipeline group (half a row)
            with loop_ctx:
              for _ in range(repeat):
                # software pipeline over groups of GRP tiles: stage 1
                # (loads, squares, s_va, norms) for group g is emitted
                # before stage 2 (diag build + PE accumulate) of group
                # g-1, so the in-order DVE stream never head-of-line
                # blocks on ACT finishing a group's squares.
                groups = [(b, tcs) for b in range(B_LOC)
                          for tcs in range(0, TCH, GRP)]
                pend = None   # (vt_s, at_s, b, tcs) awaiting stage 2
                supers = None
                for g in range(len(groups) + 1):
                    if g < len(groups):
                        b, tcs = groups[g]
                        if tcs == 0:
                            # one 512KB DMA per (b, tensor): [128, 8, 256]
                            # bf16 supertile, (p, j, d) = x[b, j*128+p, d].
                            vt_s = io_pool.tile([P, TCH, D], BF16, tag="vt")
                            at_s = io_pool.tile([P, TCH, D], BF16, tag="at")
                            vr = v[b].rearrange("(j p) d -> p j d", p=P)
                            ar = a[b].rearrange("(j p) d -> p j d", p=P)
                            nspl = 2 if b == 0 else 1
                            hh = TCH // nspl
                            for s_ in range(nspl):
                                nc.sync.dma_start(
                                    out=vt_s[:, s_ * hh:(s_ + 1) * hh, :],
                                    in_=vr[:, s_ * hh:(s_ + 1) * hh, :])
                                nc.sync.dma_start(
                                    out=at_s[:, s_ * hh:(s_ + 1) * hh, :],
                                    in_=ar[:, s_ * hh:(s_ + 1) * hh, :])
                            supers = (vt_s, at_s)
                        vt_s, at_s = supers

                        # norm^2 pairs for this group: [128, GRP, 2]
                        pair = small.tile([P, GRP, 2], F32, tag="pair")
                        for k in range(GRP):
                            tci = tcs + k
                            idx = b * TCH + tci
                            vt = vt_s[:, tci, :]
                            at = at_s[:, tci, :]

                            # s_vv on ACT (Square in sqrt_and_others set)
                            sqv = scratch.tile([P, D], BF16, tag="sqv")
                            nc.scalar.activation(
                                out=sqv[:], in_=vt,
                                func=mybir.ActivationFunctionType.Square,
                                accum_out=pair[:, k, 0:1],
                            )
                            # s_aa: ACT or DVE depending on balance knob.
                            sqa = scratch.tile([P, D], BF16, tag="sqa")
                            if idx % SQA_ACT_DEN >= SQA_ACT_DEN - SQA_ACT_NUM:
                                nc.scalar.activation(
                                    out=sqa[:], in_=at,
                                    func=mybir.ActivationFunctionType.Square,
                                    accum_out=pair[:, k, 1:2],
                                )
                            else:
                                nc.vector.scalar_tensor_tensor(
                                    out=sqa[:], in0=at, scalar=1.0, in1=at,
                                    op0=MULT, op1=MULT,
                                    accum_out=pair[:, k, 1:2],
                                )

                            # s_va: fused (v*1)*a with accum on DVE
                            prod = scratch.tile([P, D], BF16, tag="prod")
                            nc.vector.scalar_tensor_tensor(
                                out=prod[:], in0=vt, scalar=1.0, in1=at,
                                op0=MULT, op1=MULT,
                                accum_out=sva_stat[:, idx:idx + 1],
                            )

                    if pend is not None:
                        pvt_s, pat_s, pb, ptcs = pend
                        for k in range(GRP):
                            tci = ptcs + k
                            idx = pb * TCH + tci
                            # diag(inv) tiles for the PE accumulate
                            dv = diagp.tile([P, P], BF16, tag="dv")
                            da = diagp.tile([P, P], BF16, tag="da")
                            n_act = K_DIAG_ACT
                            if (2 * tci) % 16 < n_act:
                                nc.scalar.activation(
                                    out=dv[:], in_=eye_sb[:],
                                    func=mybir.ActivationFunctionType.Copy,
                                    scale=inv_stat[:, idx, 0:1])
                            else:
                                nc.vector.tensor_scalar(
                                    out=dv[:], in0=eye_sb[:],
                                    scalar1=inv_stat[:, idx, 0:1],
                                    scalar2=None, op0=MULT)
                            if (2 * tci + 1) % 16 < n_act:
                                nc.scalar.activation(
                                    out=da[:], in_=eye_sb[:],
                                    func=mybir.ActivationFunctionType.Copy,
                                    scale=inv_stat[:, idx, 1:2])
                            else:
                                nc.vector.tensor_scalar(
                                    out=da[:], in0=eye_sb[:],
                                    scalar1=inv_stat[:, idx, 1:2],
                                    scalar2=None, op0=MULT)
                            # V_acc[:,tci,:] += inv_v * v ; A same, in PSUM.
                            # V@b0 start clears the whole bank; A@b0 lands
                            # on cleared has_written bits -> overwrite.
                            nc.tensor.matmul(
                                psum[tci][:, 0, :], dv[:],
                                pvt_s[:, tci, :],
                                start=(pb == 0), stop=False)
                            nc.tensor.matmul(
                                psum[tci][:, 1, :], da[:],
                                pat_s[:, tci, :],
                                start=False, stop=(pb == B_LOC - 1))
                        if pb == B_LOC - 1:
                            # final row: evict finished banks, A then V on
                            # one engine (A@b7 is the last PE write; the
                            # in-order engine stream keeps reads safe).
                            for k in range(GRP):
                                tci = ptcs + k
                                if K_EVICT == "act":
                                    nc.scalar.copy(
                                        out=aout[:, tci, :],
                                        in_=psum[tci][:, 1, :])
                                    nc.scalar.copy(
                                        out=vout[:, tci, :],
                                        in_=psum[tci][:, 0, :])
                                else:
                                    nc.vector.tensor_copy(
                                        aout[:, tci, :], psum[tci][:, 1, :])
                                    nc.vector.tensor_copy(
                                        vout[:, tci, :], psum[tci][:, 0, :])
                            if repeat == 1 and loop_n == 1:
                                lo, hi = ptcs * D, (ptcs + GRP) * D
                                nc.sync.dma_start(
                                    out=vacc_out[:, lo:hi],
                                    in_=vout[:, ptcs:ptcs + GRP, :])
                                nc.sync.dma_start(
                                    out=aacc_out[:, lo:hi],
                                    in_=aout[:, ptcs:ptcs + GRP, :])

                    if g < len(groups):
                        # batched norm + reciprocal for this group
                        norm = small.tile([P, GRP, 2], F32, tag="norm")
                        nc.scalar.activation(
                            out=norm[:], in_=pair[:],
                            func=mybir.ActivationFunctionType.Sqrt,
                        )
                        i0 = b * TCH + tcs
                        # ~51-ULP approx reciprocal, ~5x faster than the
                        # bit-exact iterative divide; plenty for a loss fn.
                        nc.vector.reciprocal_approx_fast(
                            out=inv_stat[:, i0:i0 + GRP, :], in_=norm[:])
                        pend = (vt_s, at_s, b, tcs)
                    else:
                        pend = None

            # cos = s_va * inv_v * inv_a  (deferred, two [128,64] DVE ops)
            ii = accp.tile([P, NTILES], F32)
            nc.vector.tensor_mul(
                out=ii[:], in0=inv_stat[:, :, 0], in1=inv_stat[:, :, 1]
            )
            nc.vector.tensor_mul(out=cos_stat[:], in0=ii[:], in1=sva_stat[:])

            nc.sync.dma_start(out=cos_out[:, :], in_=cos_stat[:])
            if repeat != 1 or loop_n != 1:
                nc.sync.dma_start(out=vacc_out[:, :], in_=vout[:])
                nc.sync.dma_start(out=aacc_out[:, :], in_=aout[:])

    nc.compile()
    return nc


def _get_nc(repeat=1, loop_n=1):
    key = ("nc", repeat, loop_n)
    if key not in _CACHE:
        _CACHE[key] = _build_nc(repeat, loop_n)
    return _CACHE[key]


_EYE = np.eye(P, dtype=ml_dtypes.bfloat16)


def _run(nc, f_v, f_a):
    in_maps = [
        {
            "v": np.ascontiguousarray(f_v[c * B_LOC:(c + 1) * B_LOC]),
            "a": np.ascontiguousarray(f_a[c * B_LOC:(c + 1) * B_LOC]),
            "eye": _EYE,
        }
        for c in range(N_CORES)
    ]
    return run_bass_kernel_spmd(nc, in_maps, core_ids=list(range(N_CORES)))


def kernel(f_v, f_a, labels):
    global LAST_RESULTS
    f_v = np.asarray(f_v, dtype=np.float32).astype(ml_dtypes.bfloat16)
    f_a = np.asarray(f_a, dtype=np.float32).astype(ml_dtypes.bfloat16)
    labels = np.asarray(labels)

    res = _run(_get_nc(), f_v, f_a)
    LAST_RESULTS = res
    out = res.results

    # cos_stat[c][p, b_loc*TCH+tc] = cos(b=c*B_LOC+b_loc, t=tc*128+p)
    cos = np.stack([out[c]["cos_stat"] for c in range(N_CORES)])
    cos = cos.reshape(N_CORES, P, B_LOC, TCH)
    row_cos = cos.sum(axis=(1, 3), dtype=np.float64).reshape(B)

    v_acc = np.zeros((P, TCH * D), np.float64)
    a_acc = np.zeros((P, TCH * D), np.float64)
    for c in range(N_CORES):
        v_acc += out[c]["v_acc"]
        a_acc += out[c]["a_acc"]
    cross_sum = float((v_acc * a_acc).sum())   # = sum_t V_t . A_t

    pos = labels == 0
    n_pos = int(pos.sum())
    n_neg = B - n_pos

    loss_pos = ALPHA * (n_pos * T - row_cos[pos].sum())
    loss_neg = BETA * row_cos[~pos].sum()
    loss_neg += GAMMA * (cross_sum - row_cos.sum()) / T
    cnt_pos = n_pos * T
    cnt_neg = n_neg * T + B * (B - 1)

    loss = 0.0
    if cnt_pos > 0:
        loss += loss_pos / max(cnt_pos, 1.0)
    if cnt_neg > 0:
        loss += loss_neg / max(cnt_neg, 1.0)
    return np.float32(loss)
